# revision 1
# baseline (speedup 1.0000x reference)
"""Single-head attention (InterModalAttention) Bass kernel for 8 TRN2 cores.

Sharding: batch (4) x query-half (2) -> 8 cores. Each core computes K/V for
its batch element (full 2048-seq) and attention for its 1024 queries.

Layout strategy (all matmuls contract over the partition dim):
  - Host pre-transposes x and weights -> xT [d,s], WT [d,e] so no on-chip
    transpose of inputs is needed.
  - qT/kT computed as [e, s] tiles (lhsT=WT tile, rhs=xT tile); bias added
    per-partition during PSUM->SBUF copyback.
  - v computed natural [s, e] (lhsT=xT tile, rhs=WvT tile); bias bv folded
    into the final epilogue (softmax rows sum to 1).
  - scores[i,j] psum accumulated over 8 e-tiles; exp on ACT engine with
    scale=1/32 and accum_out giving row-sums for free.
  - attn tiles PE-transposed (128x128) -> lhsT for out = attnT.T @ v,
    accumulated over 16 j-tiles in PSUM.
  - epilogue: out = psum * (1/rowsum) + bv.
All matmul operands use float32r (full-rate fp32 on the PE at N>=512).
"""
import sys
import numpy as np

for p in ("/opt/trn_rl_repo",):
    if p not in sys.path:
        sys.path.insert(0, p)

B, S, D = 4, 2048, 1024
NQ = 1024          # queries per core
NCORES = 8
P = 128
INV_SQRT_D = 1.0 / 32.0

_CACHE = {}


def build_nc():
    from contextlib import ExitStack
    import concourse.mybir as mybir
    import concourse.tile as tile
    from concourse import bacc
    from concourse.masks import make_identity

    F32 = mybir.dt.float32
    FR = mybir.dt.float32r
    AF = mybir.ActivationFunctionType

    nc = bacc.Bacc("TRN2", debug=False)

    xkvT = nc.dram_tensor("xkvT", (D, S), FR, kind="ExternalInput")
    xqT = nc.dram_tensor("xqT", (D, NQ), FR, kind="ExternalInput")
    wqT = nc.dram_tensor("wqT", (D, D), FR, kind="ExternalInput")
    wkT = nc.dram_tensor("wkT", (D, D), FR, kind="ExternalInput")
    wvT = nc.dram_tensor("wvT", (D, D), FR, kind="ExternalInput")
    bq = nc.dram_tensor("bq", (D,), F32, kind="ExternalInput")
    bk = nc.dram_tensor("bk", (D,), F32, kind="ExternalInput")
    bv = nc.dram_tensor("bv", (D,), F32, kind="ExternalInput")
    out = nc.dram_tensor("out", (NQ, D), F32, kind="ExternalOutput")

    ET = D // P            # 8 e-tiles
    DT = D // P            # 8 d-tiles
    SC = S // 512          # 4 s-chunks
    SB = S // P            # 16 s-blocks (j-tiles)
    IG = NQ // 512         # 2 i-groups
    EC = D // 512          # 2 e-chunks

    with tile.TileContext(nc) as tc, ExitStack() as ctx:
        consts = ctx.enter_context(tc.tile_pool(name="consts", bufs=1))
        ps512 = ctx.enter_context(tc.tile_pool(name="ps512", bufs=2, space="PSUM"))
        outps = ctx.enter_context(tc.tile_pool(name="outps", bufs=2, space="PSUM"))
        tpps = ctx.enter_context(tc.tile_pool(name="tpps", bufs=2, space="PSUM"))
        dram = ctx.enter_context(tc.tile_pool(name="dram", bufs=1, space="DRAM"))

        _eng = [nc.sync, nc.gpsimd, nc.scalar]
        _dmac = [0]
        def dma(out_ap, in_ap):
            e = _eng[_dmac[0] % len(_eng)]
            _dmac[0] += 1
            e.dma_start(out_ap, in_ap)

        # ---- constants ----
        ident_f = consts.tile([P, P], F32)
        make_identity(nc, ident_f)
        ident = consts.tile([P, P], FR)
        nc.gpsimd.dma_start(ident[:], ident_f[:])

        ones_f = consts.tile([1, P], F32)
        nc.gpsimd.memset(ones_f[:], 1.0)
        ones = consts.tile([1, P], FR)
        nc.gpsimd.dma_start(ones[:], ones_f[:])

        bv_sb = consts.tile([1, D], FR)
        nc.gpsimd.dma_start(bv_sb[:], bv[:].rearrange("(one d) -> one d", one=1))
        bq_sb = consts.tile([P, ET], F32)
        nc.sync.dma_start(bq_sb[:], bq[:].rearrange("(t p) -> p t", p=P))
        bk_sb = consts.tile([P, ET], F32)
        nc.sync.dma_start(bk_sb[:], bk[:].rearrange("(t p) -> p t", p=P))

        # bv broadcast to [P, D] via ones.T @ bv (K=1 matmul)
        bv_bcast = consts.tile([P, D], F32)
        for ec in range(EC):
            pstmp = ps512.tile([P, 512], F32, tag="ps512")
            nc.tensor.matmul(pstmp[:], ones[:], bv_sb[:, ec * 512:(ec + 1) * 512],
                             start=True, stop=True)
            nc.any.tensor_copy(bv_bcast[:, ec * 512:(ec + 1) * 512], pstmp[:])

        qT_dram = dram.tile([D, NQ], FR)
        kpool = ctx.enter_context(tc.tile_pool(name="kpool", bufs=1))
        kT = kpool.tile([P, ET, S], FR)      # [e-part, e-tile, j]

        # ---- Phase 1: Q projection (wk prefetched) ----
        wk_ctx = tc.tile_pool(name="wk", bufs=1)
        wkp = wk_ctx.__enter__()
        with tc.tile_pool(name="wq", bufs=1) as wqp, \
             tc.tile_pool(name="xq", bufs=1) as xqp, \
             tc.tile_pool(name="qo", bufs=2) as qop:
            wq_sb = wqp.tile([P, DT, D], FR)
            for dt in range(DT):
                dma(wq_sb[:, dt, :], wqT[dt * P:(dt + 1) * P, :])
            wk_sb = wkp.tile([P, DT, D], FR)
            for dt in range(DT):
                dma(wk_sb[:, dt, :], wkT[dt * P:(dt + 1) * P, :])
            for g in range(IG):
                xq_g = xqp.tile([P, DT, 512], FR, tag="xq")
                for dt in range(DT):
                    dma(xq_g[:, dt, :],
                                      xqT[dt * P:(dt + 1) * P, g * 512:(g + 1) * 512])
                for et in range(ET):
                    psq = ps512.tile([P, 512], F32, tag="ps512")
                    for dt in range(DT):
                        nc.tensor.matmul(psq[:], wq_sb[:, dt, et * P:(et + 1) * P],
                                         xq_g[:, dt, :], start=(dt == 0), stop=(dt == DT - 1))
                    qo = qop.tile([P, 512], FR, tag="qo")
                    nc.vector.tensor_scalar_add(qo[:], psq[:], bq_sb[:, et:et + 1])
                    dma(qT_dram[et * P:(et + 1) * P, g * 512:(g + 1) * 512], qo[:])

        # ---- Phase 2: K projection -> kT resident [e, j] ----
        with tc.tile_pool(name="xk", bufs=2) as xkp:
            for sc in range(SC):
                xk_g = xkp.tile([P, DT, 512], FR, tag="xk")
                for dt in range(DT):
                    dma(xk_g[:, dt, :],
                                      xkvT[dt * P:(dt + 1) * P, sc * 512:(sc + 1) * 512])
                for et in range(ET):
                    psk = ps512.tile([P, 512], F32, tag="ps512")
                    for dt in range(DT):
                        nc.tensor.matmul(psk[:], wk_sb[:, dt, et * P:(et + 1) * P],
                                         xk_g[:, dt, :], start=(dt == 0), stop=(dt == DT - 1))
                    nc.vector.tensor_scalar_add(kT[:, et, sc * 512:(sc + 1) * 512],
                                                psk[:], bk_sb[:, et:et + 1])

        # ---- Phase 3: V projection -> v resident [j, e] (no bias) ----
        wk_ctx.__exit__(None, None, None)
        vpool = ctx.enter_context(tc.tile_pool(name="vpool", bufs=1))
        vN = vpool.tile([P, SB, D], FR)      # [s-part, j-tile, e]
        with tc.tile_pool(name="wv", bufs=1) as wvp, \
             tc.tile_pool(name="xv", bufs=2) as xvp:
            wv_sb = wvp.tile([P, DT, D], FR)
            for dt in range(DT):
                dma(wv_sb[:, dt, :], wvT[dt * P:(dt + 1) * P, :])
            for sb_i in range(SB):
                xv_g = xvp.tile([P, DT, P], FR, tag="xv")
                for dt in range(DT):
                    dma(xv_g[:, dt, :],
                                      xkvT[dt * P:(dt + 1) * P, sb_i * P:(sb_i + 1) * P])
                for ec in range(EC):
                    psv = ps512.tile([P, 512], F32, tag="ps512")
                    for dt in range(DT):
                        nc.tensor.matmul(psv[:], xv_g[:, dt, :],
                                         wv_sb[:, dt, ec * 512:(ec + 1) * 512],
                                         start=(dt == 0), stop=(dt == DT - 1))
                    nc.any.tensor_copy(vN[:, sb_i, ec * 512:(ec + 1) * 512], psv[:])

        # ---- Phase 4: attention ----
        with tc.tile_pool(name="qg", bufs=1) as qgp, \
             tc.tile_pool(name="attn", bufs=3) as attnp, \
             tc.tile_pool(name="attnT", bufs=6) as attnTp, \
             tc.tile_pool(name="epi", bufs=2) as epip:
            for g in range(IG):
                qT_g = qgp.tile([P, ET, 512], FR, tag="qg")
                for et in range(ET):
                    dma(qT_g[:, et, :],
                                      qT_dram[et * P:(et + 1) * P, g * 512:(g + 1) * 512])
                for ib in range(4):
                    i0 = ib * P
                    out_ps = [outps.tile([P, 512], F32, tag=f"outps{ec}", name=f"out_ps{ec}")
                              for ec in range(EC)]
                    rs = epip.tile([P, SC], F32, tag="rs")
                    for jc in range(SC):
                        sc_ps = ps512.tile([P, 512], F32, tag="ps512")
                        for et in range(ET):
                            nc.tensor.matmul(sc_ps[:], qT_g[:, et, i0:i0 + P],
                                             kT[:, et, jc * 512:(jc + 1) * 512],
                                             start=(et == 0), stop=(et == ET - 1))
                        attn = attnp.tile([P, 512], FR, tag="attn")
                        nc.scalar.activation(attn[:], sc_ps[:], AF.Exp,
                                             scale=INV_SQRT_D, accum_out=rs[:, jc:jc + 1])
                        for jt in range(4):
                            jg = jc * 4 + jt
                            tps = tpps.tile([P, P], FR, tag="tps")
                            nc.tensor.transpose(tps[:], attn[:, jt * P:(jt + 1) * P], ident[:])
                            attnT = attnTp.tile([P, P], FR, tag="attnT")
                            nc.any.tensor_copy(attnT[:], tps[:])
                            for ec in range(EC):
                                nc.tensor.matmul(out_ps[ec][:], attnT[:],
                                                 vN[:, jg, ec * 512:(ec + 1) * 512],
                                                 start=(jg == 0), stop=(jg == SB - 1))
                    rsum = epip.tile([P, 1], F32, tag="rsum")
                    nc.vector.tensor_reduce(rsum[:], rs[:], mybir.AxisListType.X,
                                            mybir.AluOpType.add)
                    invs = epip.tile([P, 1], F32, tag="invs")
                    nc.vector.reciprocal(invs[:], rsum[:])
                    out_sb = epip.tile([P, D], F32, tag="out_sb")
                    for ec in range(EC):
                        nc.vector.tensor_scalar_mul(out_sb[:, ec * 512:(ec + 1) * 512],
                                                    out_ps[ec][:], invs[:])
                    nc.vector.tensor_add(out_sb[:], out_sb[:], bv_bcast[:])
                    r0 = g * 512 + i0
                    dma(out[r0:r0 + P, :], out_sb[:])

    nc.compile()
    return nc


def make_in_maps(x, Wq, bq, Wk, bk, Wv, bv):
    x = np.asarray(x, np.float32)
    wqT = np.ascontiguousarray(np.asarray(Wq, np.float32).T)
    wkT = np.ascontiguousarray(np.asarray(Wk, np.float32).T)
    wvT = np.ascontiguousarray(np.asarray(Wv, np.float32).T)
    bq = np.ascontiguousarray(np.asarray(bq, np.float32))
    bk = np.ascontiguousarray(np.asarray(bk, np.float32))
    bv = np.ascontiguousarray(np.asarray(bv, np.float32))
    in_maps = []
    for c in range(NCORES):
        b, h = c // 2, c % 2
        xb = x[b]
        in_maps.append({
            "xkvT": np.ascontiguousarray(xb.T),
            "xqT": np.ascontiguousarray(xb[h * NQ:(h + 1) * NQ].T),
            "wqT": wqT, "wkT": wkT, "wvT": wvT,
            "bq": bq, "bk": bk, "bv": bv,
        })
    return in_maps


def get_nc():
    if "nc" not in _CACHE:
        _CACHE["nc"] = build_nc()
    return _CACHE["nc"]


def kernel(x, Wq, bq, Wk, bk, Wv, bv):
    from concourse.bass_utils import run_bass_kernel_spmd
    nc = get_nc()
    in_maps = make_in_maps(x, Wq, bq, Wk, bk, Wv, bv)
    res = run_bass_kernel_spmd(nc, in_maps, core_ids=list(range(NCORES)))
    out = np.empty((B, S, D), np.float32)
    for c in range(NCORES):
        b, h = c // 2, c % 2
        out[b, h * NQ:(h + 1) * NQ] = res.results[c]["out"]
    return out



# revision 3
# speedup vs baseline: 1.3233x; 1.3233x over previous
"""Single-head attention (InterModalAttention) Bass kernel for 8 TRN2 cores.

Sharding: batch (4) x query-half (2) -> 8 cores. Each core computes K/V for
its batch element (full 2048-seq) and attention for its 1024 queries.

v2 layout strategy (all bf16 matmul operands, f32 PSUM accumulation):
  - Host pre-transposes x and weights and rotates x columns per core so the
    core's query half occupies columns 0:1024 of xT -- no separate xq input.
  - x, weights, qT, kT, vN all SBUF-resident; no DRAM roundtrips.
  - qT/kT computed as [e, s] tiles (lhsT=WT tile, rhs=xT tile); bias added
    per-partition during PSUM->SBUF copyback (f32 -> bf16).
  - v computed natural [s, e]; bias bv folded into the final epilogue.
  - scoresT [j, i] computed directly (lhsT=kT tile, rhs=qT): the exp output
    written by ACT to SBUF is already the lhsT for out = attnT.T @ v, so no
    PE transposes and no PSUM->SBUF attn copies are needed.
  - softmax row sums come from an extra N=1 matmul against a ones column,
    reusing the stationary attnT tile already loaded for the AV matmuls.
  - epilogue: out = psum * (1/rowsum) + bv (DVE, f32).
"""
import sys
import numpy as np
import ml_dtypes

for p in ("/opt/trn_rl_repo",):
    if p not in sys.path:
        sys.path.insert(0, p)

B, S, D = 4, 2048, 1024
NQ = 1024          # queries per core
NCORES = 8
P = 128
INV_SQRT_D = 1.0 / 32.0
BF16 = ml_dtypes.bfloat16

_CACHE = {}


def build_nc():
    from contextlib import ExitStack
    import concourse.mybir as mybir
    import concourse.tile as tile
    from concourse import bacc

    F32 = mybir.dt.float32
    FR = mybir.dt.float32r
    BF = mybir.dt.bfloat16
    AF = mybir.ActivationFunctionType

    nc = bacc.Bacc("TRN2", debug=False)

    xT = nc.dram_tensor("xT", (D, S), BF, kind="ExternalInput")
    wqT = nc.dram_tensor("wqT", (D, D), BF, kind="ExternalInput")
    wkT = nc.dram_tensor("wkT", (D, D), BF, kind="ExternalInput")
    wvT = nc.dram_tensor("wvT", (D, D), BF, kind="ExternalInput")
    bq = nc.dram_tensor("bq", (D,), F32, kind="ExternalInput")
    bk = nc.dram_tensor("bk", (D,), F32, kind="ExternalInput")
    bv = nc.dram_tensor("bv", (D,), F32, kind="ExternalInput")
    out = nc.dram_tensor("out", (NQ, D), F32, kind="ExternalOutput")

    DT = D // P            # 8 d-tiles (contraction for projections)
    ET = D // P            # 8 e-tiles
    JT = S // P            # 16 j-tiles
    JC = S // 512          # 4 j-chunks
    IG = NQ // 512         # 2 i-groups
    EC = D // 512          # 2 e-chunks

    with tile.TileContext(nc) as tc, ExitStack() as ctx:
        consts = ctx.enter_context(tc.tile_pool(name="consts", bufs=1))
        ps512 = ctx.enter_context(tc.tile_pool(name="ps512", bufs=2, space="PSUM"))
        outps = ctx.enter_context(tc.tile_pool(name="outps", bufs=2, space="PSUM"))
        qpool = ctx.enter_context(tc.tile_pool(name="qpool", bufs=1))
        kpool = ctx.enter_context(tc.tile_pool(name="kpool", bufs=1))
        vpool = ctx.enter_context(tc.tile_pool(name="vpool", bufs=1))

        _eng = [nc.sync, nc.gpsimd]
        _dmac = [0]
        def dma(out_ap, in_ap):
            e = _eng[_dmac[0] % len(_eng)]
            _dmac[0] += 1
            e.dma_start(out_ap, in_ap)

        # ---- constants ----
        ones_f = consts.tile([P, 1], F32)
        nc.gpsimd.memset(ones_f[:], 1.0)
        ones_col = consts.tile([P, 1], BF)
        nc.vector.tensor_copy(ones_col[:], ones_f[:])

        ones_row_f = consts.tile([1, P], F32)
        nc.gpsimd.memset(ones_row_f[:], 1.0)
        ones_row = consts.tile([1, P], FR)
        nc.gpsimd.dma_start(ones_row[:], ones_row_f[:])
        bv_sb = consts.tile([1, D], FR)
        nc.gpsimd.dma_start(bv_sb[:], bv[:].rearrange("(one d) -> one d", one=1))
        bq_sb = consts.tile([P, ET], F32)
        nc.sync.dma_start(bq_sb[:], bq[:].rearrange("(t p) -> p t", p=P))
        bk_sb = consts.tile([P, ET], F32)
        nc.sync.dma_start(bk_sb[:], bk[:].rearrange("(t p) -> p t", p=P))

        # bv broadcast to [P, D] via ones.T @ bv (K=1 matmul)
        bv_bcast = consts.tile([P, D], F32)
        for ec in range(EC):
            pstmp = ps512.tile([P, 512], F32, tag="ps512")
            nc.tensor.matmul(pstmp[:], ones_row[:], bv_sb[:, ec * 512:(ec + 1) * 512],
                             start=True, stop=True)
            nc.any.tensor_copy(bv_bcast[:, ec * 512:(ec + 1) * 512], pstmp[:])

        qT_sb = qpool.tile([P, ET, NQ], BF)      # [e-part, e-tile, i]
        kT_sb = kpool.tile([P, ET, S], BF)       # [e-part, e-tile, j]
        vN_sb = vpool.tile([P, JT, D], BF)       # [j-part, j-tile, e]

        # ---- projections ----
        with tc.tile_pool(name="wq", bufs=1) as wqp, \
             tc.tile_pool(name="wk", bufs=1) as wkp, \
             tc.tile_pool(name="wv", bufs=1) as wvp, \
             tc.tile_pool(name="xp", bufs=1) as xp:
            wq_sb = wqp.tile([P, DT, D], BF)
            wk_sb = wkp.tile([P, DT, D], BF)
            wv_sb = wvp.tile([P, DT, D], BF)
            xT_sb = xp.tile([P, DT, S], BF)
            # DMA issue order: Q-path first so the PE can start ASAP.
            for dt in range(DT):
                dma(wq_sb[:, dt, :], wqT[dt * P:(dt + 1) * P, :])
                dma(xT_sb[:, dt, 0:NQ], xT[dt * P:(dt + 1) * P, 0:NQ])
            for dt in range(DT):
                dma(wk_sb[:, dt, :], wkT[dt * P:(dt + 1) * P, :])
                dma(xT_sb[:, dt, NQ:S], xT[dt * P:(dt + 1) * P, NQ:S])
            for dt in range(DT):
                dma(wv_sb[:, dt, :], wvT[dt * P:(dt + 1) * P, :])

            # Q projection: qT[e, i]
            for g in range(IG):
                for et in range(ET):
                    ps = ps512.tile([P, 512], F32, tag="ps512")
                    for dt in range(DT):
                        nc.tensor.matmul(ps[:], wq_sb[:, dt, et * P:(et + 1) * P],
                                         xT_sb[:, dt, g * 512:(g + 1) * 512],
                                         start=(dt == 0), stop=(dt == DT - 1))
                    nc.vector.tensor_scalar_add(qT_sb[:, et, g * 512:(g + 1) * 512],
                                                ps[:], bq_sb[:, et:et + 1])
            # K projection: kT[e, j]
            for jc in range(JC):
                for et in range(ET):
                    ps = ps512.tile([P, 512], F32, tag="ps512")
                    for dt in range(DT):
                        nc.tensor.matmul(ps[:], wk_sb[:, dt, et * P:(et + 1) * P],
                                         xT_sb[:, dt, jc * 512:(jc + 1) * 512],
                                         start=(dt == 0), stop=(dt == DT - 1))
                    nc.vector.tensor_scalar_add(kT_sb[:, et, jc * 512:(jc + 1) * 512],
                                                ps[:], bk_sb[:, et:et + 1])
            # V projection: v[j, e] (no bias; folded into epilogue)
            for jt in range(JT):
                for ec in range(EC):
                    ps = ps512.tile([P, 512], F32, tag="ps512")
                    for dt in range(DT):
                        nc.tensor.matmul(ps[:], xT_sb[:, dt, jt * P:(jt + 1) * P],
                                         wv_sb[:, dt, ec * 512:(ec + 1) * 512],
                                         start=(dt == 0), stop=(dt == DT - 1))
                    nc.any.tensor_copy(vN_sb[:, jt, ec * 512:(ec + 1) * 512], ps[:])

        # ---- attention ----
        with tc.tile_pool(name="attn", bufs=2) as attnp, \
             tc.tile_pool(name="epi", bufs=2) as epip:
            for g in range(IG):
                aT = attnp.tile([P, JT, 512], BF, tag="aT")   # [j-part, j-tile, i]
                for jt in range(JT):
                    ps = ps512.tile([P, 512], F32, tag="ps512")
                    for et in range(ET):
                        nc.tensor.matmul(ps[:], kT_sb[:, et, jt * P:(jt + 1) * P],
                                         qT_sb[:, et, g * 512:(g + 1) * 512],
                                         start=(et == 0), stop=(et == ET - 1))
                    nc.scalar.activation(aT[:, jt, :], ps[:], AF.Exp, scale=INV_SQRT_D)
                for ib in range(4):
                    o0 = outps.tile([P, 512], F32, tag="o0")
                    o1 = outps.tile([P, 512], F32, tag="o1")
                    rs = outps.tile([P, 1], F32, tag="rs")
                    for jt in range(JT):
                        lhsT = aT[:, jt, ib * P:(ib + 1) * P]
                        st, sp = (jt == 0), (jt == JT - 1)
                        nc.tensor.matmul(o0[:], lhsT, vN_sb[:, jt, 0:512],
                                         start=st, stop=sp)
                        nc.tensor.matmul(o1[:], lhsT, vN_sb[:, jt, 512:1024],
                                         start=st, stop=sp)
                        nc.tensor.matmul(rs[:], lhsT, ones_col[:],
                                         start=st, stop=sp)
                    inv = epip.tile([P, 1], F32, tag="inv")
                    nc.vector.reciprocal(inv[:], rs[:])
                    osb = epip.tile([P, D], F32, tag="osb")
                    nc.vector.tensor_scalar_mul(osb[:, 0:512], o0[:], inv[:])
                    nc.vector.tensor_scalar_mul(osb[:, 512:1024], o1[:], inv[:])
                    nc.vector.tensor_add(osb[:], osb[:], bv_bcast[:])
                    r0 = g * 512 + ib * P
                    dma(out[r0:r0 + P, :], osb[:])

    nc.compile()
    return nc


def make_in_maps(x, Wq, bq, Wk, bk, Wv, bv):
    x = np.asarray(x, np.float32)
    wqT = np.asarray(Wq, np.float32).T.astype(BF16)
    wkT = np.asarray(Wk, np.float32).T.astype(BF16)
    wvT = np.asarray(Wv, np.float32).T.astype(BF16)
    bq = np.ascontiguousarray(np.asarray(bq, np.float32))
    bk = np.ascontiguousarray(np.asarray(bk, np.float32))
    bv = np.ascontiguousarray(np.asarray(bv, np.float32))
    in_maps = []
    for c in range(NCORES):
        b, h = c // 2, c % 2
        xb = x[b]
        # rotate so this core's query half is columns 0:NQ of xT
        xrot = np.concatenate([xb[h * NQ:(h + 1) * NQ], xb[(1 - h) * NQ:(2 - h) * NQ]], axis=0)
        in_maps.append({
            "xT": xrot.T.astype(BF16),
            "wqT": wqT, "wkT": wkT, "wvT": wvT,
            "bq": bq, "bk": bk, "bv": bv,
        })
    return in_maps


def get_nc():
    if "nc" not in _CACHE:
        _CACHE["nc"] = build_nc()
    return _CACHE["nc"]


def kernel(x, Wq, bq, Wk, bk, Wv, bv):
    from concourse.bass_utils import run_bass_kernel_spmd
    nc = get_nc()
    in_maps = make_in_maps(x, Wq, bq, Wk, bk, Wv, bv)
    res = run_bass_kernel_spmd(nc, in_maps, core_ids=list(range(NCORES)))
    out = np.empty((B, S, D), np.float32)
    for c in range(NCORES):
        b, h = c // 2, c % 2
        out[b, h * NQ:(h + 1) * NQ] = res.results[c]["out"]
    return out


# revision 17
# speedup vs baseline: 1.5044x; 1.1368x over previous
"""Single-head attention (InterModalAttention) Bass kernel for 8 TRN2 cores.

Sharding: batch (4) x query-half (2) -> 8 cores. Each core computes K/V for
its batch element (full 2048-seq) and attention for its 1024 queries.

v2 layout strategy (all bf16 matmul operands, f32 PSUM accumulation):
  - Host pre-transposes x and weights and rotates x columns per core so the
    core's query half occupies columns 0:1024 of xT -- no separate xq input.
  - x, weights, qT, kT, vN all SBUF-resident; no DRAM roundtrips.
  - qT/kT computed as [e, s] tiles (lhsT=WT tile, rhs=xT tile); bias added
    per-partition during PSUM->SBUF copyback (f32 -> bf16).
  - v computed natural [s, e]; bias bv folded into the final epilogue.
  - scoresT [j, i] computed directly (lhsT=kT tile, rhs=qT): the exp output
    written by ACT to SBUF is already the lhsT for out = attnT.T @ v, so no
    PE transposes and no PSUM->SBUF attn copies are needed.
  - softmax row sums come from an extra N=1 matmul against a ones column,
    reusing the stationary attnT tile already loaded for the AV matmuls.
  - epilogue: out = psum * (1/rowsum) + bv (DVE, f32).
"""
import sys
import numpy as np
import ml_dtypes

for p in ("/opt/trn_rl_repo",):
    if p not in sys.path:
        sys.path.insert(0, p)

B, S, D = 4, 2048, 1024
NQ = 1024          # queries per core
NCORES = 8
P = 128
INV_SQRT_D = 1.0 / 32.0
BF16 = ml_dtypes.bfloat16

_CACHE = {}


def build_nc():
    from contextlib import ExitStack
    import concourse.mybir as mybir
    import concourse.tile as tile
    from concourse import bacc

    F32 = mybir.dt.float32
    FR = mybir.dt.float32r
    BF = mybir.dt.bfloat16
    F8 = mybir.dt.float8e4
    DR = mybir.MatmulPerfMode.DoubleRow
    AF = mybir.ActivationFunctionType

    nc = bacc.Bacc("TRN2", debug=False)

    xT = nc.dram_tensor("xT", (D, S), BF, kind="ExternalInput")
    wqT = nc.dram_tensor("wqT", (D, D), BF, kind="ExternalInput")
    wkT = nc.dram_tensor("wkT", (D, D), BF, kind="ExternalInput")
    wvT = nc.dram_tensor("wvT", (D, D), BF, kind="ExternalInput")
    bq = nc.dram_tensor("bq", (D,), F32, kind="ExternalInput")
    bk = nc.dram_tensor("bk", (D,), F32, kind="ExternalInput")
    bv = nc.dram_tensor("bv", (D,), F32, kind="ExternalInput")
    out = nc.dram_tensor("out", (NQ, D), F32, kind="ExternalOutput")

    DT = D // P            # 8 d-tiles (contraction for projections)
    ET = D // P            # 8 e-tiles
    JT = S // P            # 16 j-tiles
    JC = S // 512          # 4 j-chunks
    IG = NQ // 512         # 2 i-groups
    EC = D // 512          # 2 e-chunks

    with tile.TileContext(nc) as tc, ExitStack() as ctx:
        consts = ctx.enter_context(tc.tile_pool(name="consts", bufs=1))
        ps512 = ctx.enter_context(tc.tile_pool(name="ps512", bufs=2, space="PSUM"))
        outps = ctx.enter_context(tc.tile_pool(name="outps", bufs=2, space="PSUM"))
        qpool = ctx.enter_context(tc.tile_pool(name="qpool", bufs=1))
        kpool = ctx.enter_context(tc.tile_pool(name="kpool", bufs=1))
        vpool = ctx.enter_context(tc.tile_pool(name="vpool", bufs=1))

        _eng = [nc.sync, nc.gpsimd]
        _dmac = [0]
        def dma(out_ap, in_ap):
            e = _eng[_dmac[0] % len(_eng)]
            _dmac[0] += 1
            e.dma_start(out_ap, in_ap)

        # ---- constants (DMAs on the scalar queue so they don't queue
        # behind the bulk weight/x loads) ----
        ones_f = consts.tile([P, 1], F32)
        nc.gpsimd.memset(ones_f[:], 1.0)
        ones_col = consts.tile([P, 1], BF)
        nc.vector.tensor_copy(ones_col[:], ones_f[:])

        nbias = consts.tile([P, 1], F32)
        nc.gpsimd.memset(nbias[:], -2.5)

        ones_row_f = consts.tile([1, P], F32)
        nc.gpsimd.memset(ones_row_f[:], 1.0)
        ones_row = consts.tile([1, P], FR)
        nc.gpsimd.dma_start(ones_row[:], ones_row_f[:])
        bv_sb = consts.tile([1, D], FR)
        nc.gpsimd.dma_start(bv_sb[:], bv[:].rearrange("(one d) -> one d", one=1))
        bq_sb = consts.tile([P, ET], F32)
        nc.scalar.dma_start(bq_sb[:], bq[:].rearrange("(t p) -> p t", p=P))
        bk_sb = consts.tile([P, ET], F32)
        nc.scalar.dma_start(bk_sb[:], bk[:].rearrange("(t p) -> p t", p=P))

        # bv broadcast to [P, D] via ones.T @ bv (K=1 matmul)
        bv_bcast = consts.tile([P, D], F32)
        for ec in range(EC):
            pstmp = ps512.tile([P, 512], F32, tag="ps512")
            nc.tensor.matmul(pstmp[:], ones_row[:], bv_sb[:, ec * 512:(ec + 1) * 512],
                             start=True, stop=True)
            nc.any.tensor_copy(bv_bcast[:, ec * 512:(ec + 1) * 512], pstmp[:])

        # fp8 with paired contraction layout for DoubleRow matmuls:
        # global index = partition + 128*pair + 256*tile
        qT_sb = qpool.tile([P, ET // 2, 2, NQ], F8)   # [e-part, e-tile2, e-pair, i]
        kT_sb = kpool.tile([P, ET // 2, 2, S], F8)    # [e-part, e-tile2, e-pair, j]
        vN_sb = vpool.tile([P, JT, D], BF)            # [j-part, j-tile, e]

        # ---- projections ----
        with tc.tile_pool(name="wq", bufs=1) as wqp, \
             tc.tile_pool(name="wk", bufs=1) as wkp, \
             tc.tile_pool(name="wv", bufs=1) as wvp, \
             tc.tile_pool(name="xp", bufs=1) as xp:
            wq_sb = wqp.tile([P, DT, D], BF)
            wk_sb = wkp.tile([P, DT, D], BF)
            wv_sb = wvp.tile([P, DT, D], BF)
            xT_sb = xp.tile([P, DT, S], BF)
            # DMA issue order: Q-path first so the PE can start ASAP.
            for dt in range(DT):
                dma(wq_sb[:, dt, :], wqT[dt * P:(dt + 1) * P, :])
                dma(xT_sb[:, dt, 0:NQ], xT[dt * P:(dt + 1) * P, 0:NQ])
            for dt in range(DT):
                dma(wk_sb[:, dt, :], wkT[dt * P:(dt + 1) * P, :])
                dma(xT_sb[:, dt, NQ:S], xT[dt * P:(dt + 1) * P, NQ:S])
            for dt in range(DT):
                dma(wv_sb[:, dt, :], wvT[dt * P:(dt + 1) * P, :])

            # Q projection: qT[e, i]
            for g in range(IG):
                for et in range(ET):
                    ps = ps512.tile([P, 512], F32, tag="ps512")
                    for dt in range(DT):
                        nc.tensor.matmul(ps[:], wq_sb[:, dt, et * P:(et + 1) * P],
                                         xT_sb[:, dt, g * 512:(g + 1) * 512],
                                         start=(dt == 0), stop=(dt == DT - 1))
                    nc.vector.tensor_scalar_add(
                        qT_sb[:, et // 2, et % 2, g * 512:(g + 1) * 512],
                        ps[:], bq_sb[:, et:et + 1])
            # K projection: kT[e, j]
            for jc in range(JC):
                for et in range(ET):
                    ps = ps512.tile([P, 512], F32, tag="ps512")
                    for dt in range(DT):
                        nc.tensor.matmul(ps[:], wk_sb[:, dt, et * P:(et + 1) * P],
                                         xT_sb[:, dt, jc * 512:(jc + 1) * 512],
                                         start=(dt == 0), stop=(dt == DT - 1))
                    nc.vector.tensor_scalar_add(
                        kT_sb[:, et // 2, et % 2, jc * 512:(jc + 1) * 512],
                        ps[:], bk_sb[:, et:et + 1])
            # V projection: v[j, e] (no bias; folded into epilogue)
            for jt in range(JT):
                for ec in range(EC):
                    ps = ps512.tile([P, 512], F32, tag="ps512")
                    for dt in range(DT):
                        nc.tensor.matmul(ps[:], xT_sb[:, dt, jt * P:(jt + 1) * P],
                                         wv_sb[:, dt, ec * 512:(ec + 1) * 512],
                                         start=(dt == 0), stop=(dt == DT - 1))
                    nc.any.tensor_copy(vN_sb[:, jt, ec * 512:(ec + 1) * 512], ps[:])

        # ---- attention ----
        with tc.tile_pool(name="attn", bufs=2) as attnp, \
             tc.tile_pool(name="epi", bufs=2) as epip:
            for g in range(IG):
                aT = attnp.tile([P, JT, 512], BF, tag="aT")   # [j-part, j-tile, i]
                for jt in range(JT):
                    ps = ps512.tile([P, 512], F32, tag="ps512")
                    for et2 in range(ET // 2):
                        nc.tensor.matmul(ps[:], kT_sb[:, et2, :, jt * P:(jt + 1) * P],
                                         qT_sb[:, et2, :, g * 512:(g + 1) * 512],
                                         start=(et2 == 0), stop=(et2 == ET // 2 - 1),
                                         perf_mode=DR)
                    # exp(x/32 - 2.5): constant shift is softmax-invariant
                    nc.scalar.activation(aT[:, jt, :], ps[:], AF.Exp,
                                         scale=INV_SQRT_D, bias=nbias[:])
                for ib in range(4):
                    o0 = outps.tile([P, 512], F32, tag="o0")
                    o1 = outps.tile([P, 512], F32, tag="o1")
                    rs = outps.tile([P, 1], F32, tag="rs")
                    for jt in range(JT):
                        lhsT = aT[:, jt, ib * P:(ib + 1) * P]
                        st, sp = (jt == 0), (jt == JT - 1)
                        nc.tensor.matmul(o0[:], lhsT, vN_sb[:, jt, 0:512],
                                         start=st, stop=sp)
                        nc.tensor.matmul(o1[:], lhsT, vN_sb[:, jt, 512:1024],
                                         start=st, stop=sp)
                        nc.tensor.matmul(rs[:], lhsT, ones_col[:],
                                         start=st, stop=sp)
                    inv = epip.tile([P, 1], F32, tag="inv")
                    nc.vector.reciprocal(inv[:], rs[:])
                    osb = epip.tile([P, D], F32, tag="osb")
                    nc.vector.tensor_scalar_mul(osb[:, 0:512], o0[:], inv[:])
                    nc.vector.tensor_scalar_mul(osb[:, 512:1024], o1[:], inv[:])
                    nc.vector.tensor_add(osb[:], osb[:], bv_bcast[:])
                    r0 = g * 512 + ib * P
                    dma(out[r0:r0 + P, :], osb[:])

    nc.compile()
    return nc


def make_in_maps(x, Wq, bq, Wk, bk, Wv, bv):
    x = np.asarray(x, np.float32)
    wqT = np.asarray(Wq, np.float32).T.astype(BF16)
    wkT = np.asarray(Wk, np.float32).T.astype(BF16)
    wvT = np.asarray(Wv, np.float32).T.astype(BF16)
    bq = np.ascontiguousarray(np.asarray(bq, np.float32))
    bk = np.ascontiguousarray(np.asarray(bk, np.float32))
    bv = np.ascontiguousarray(np.asarray(bv, np.float32))
    in_maps = []
    for c in range(NCORES):
        b, h = c // 2, c % 2
        xb = x[b]
        # rotate so this core's query half is columns 0:NQ of xT
        xrot = np.concatenate([xb[h * NQ:(h + 1) * NQ], xb[(1 - h) * NQ:(2 - h) * NQ]], axis=0)
        in_maps.append({
            "xT": xrot.T.astype(BF16),
            "wqT": wqT, "wkT": wkT, "wvT": wvT,
            "bq": bq, "bk": bk, "bv": bv,
        })
    return in_maps


def get_nc():
    if "nc" not in _CACHE:
        _CACHE["nc"] = build_nc()
    return _CACHE["nc"]


def kernel(x, Wq, bq, Wk, bk, Wv, bv):
    from concourse.bass_utils import run_bass_kernel_spmd
    nc = get_nc()
    in_maps = make_in_maps(x, Wq, bq, Wk, bk, Wv, bv)
    res = run_bass_kernel_spmd(nc, in_maps, core_ids=list(range(NCORES)))
    out = np.empty((B, S, D), np.float32)
    for c in range(NCORES):
        b, h = c // 2, c % 2
        out[b, h * NQ:(h + 1) * NQ] = res.results[c]["out"]
    return out


# revision 20
# speedup vs baseline: 1.5077x; 1.0022x over previous
"""Single-head attention (InterModalAttention) Bass kernel for 8 TRN2 cores.

Sharding: batch (4) x query-half (2) -> 8 cores. Each core computes K/V for
its batch element (full 2048-seq) and attention for its 1024 queries.

v2 layout strategy (all bf16 matmul operands, f32 PSUM accumulation):
  - Host pre-transposes x and weights and rotates x columns per core so the
    core's query half occupies columns 0:1024 of xT -- no separate xq input.
  - x, weights, qT, kT, vN all SBUF-resident; no DRAM roundtrips.
  - qT/kT computed as [e, s] tiles (lhsT=WT tile, rhs=xT tile); bias added
    per-partition during PSUM->SBUF copyback (f32 -> bf16).
  - v computed natural [s, e]; bias bv folded into the final epilogue.
  - scoresT [j, i] computed directly (lhsT=kT tile, rhs=qT): the exp output
    written by ACT to SBUF is already the lhsT for out = attnT.T @ v, so no
    PE transposes and no PSUM->SBUF attn copies are needed.
  - softmax row sums come from an extra N=1 matmul against a ones column,
    reusing the stationary attnT tile already loaded for the AV matmuls.
  - epilogue: out = psum * (1/rowsum) + bv (DVE, f32).
"""
import sys
import numpy as np
import ml_dtypes

for p in ("/opt/trn_rl_repo",):
    if p not in sys.path:
        sys.path.insert(0, p)

B, S, D = 4, 2048, 1024
NQ = 1024          # queries per core
NCORES = 8
P = 128
INV_SQRT_D = 1.0 / 32.0
BF16 = ml_dtypes.bfloat16

_CACHE = {}


def build_nc():
    from contextlib import ExitStack
    import concourse.mybir as mybir
    import concourse.tile as tile
    from concourse import bacc

    F32 = mybir.dt.float32
    FR = mybir.dt.float32r
    BF = mybir.dt.bfloat16
    F8 = mybir.dt.float8e4
    DR = mybir.MatmulPerfMode.DoubleRow
    AF = mybir.ActivationFunctionType

    nc = bacc.Bacc("TRN2", debug=False)

    xT = nc.dram_tensor("xT", (D, S), BF, kind="ExternalInput")
    wqT = nc.dram_tensor("wqT", (D, D), BF, kind="ExternalInput")
    wkT = nc.dram_tensor("wkT", (D, D), BF, kind="ExternalInput")
    wvT = nc.dram_tensor("wvT", (D, D), BF, kind="ExternalInput")
    bq = nc.dram_tensor("bq", (D,), F32, kind="ExternalInput")
    bk = nc.dram_tensor("bk", (D,), F32, kind="ExternalInput")
    bv = nc.dram_tensor("bv", (D,), F32, kind="ExternalInput")
    out = nc.dram_tensor("out", (NQ, D), F32, kind="ExternalOutput")

    DT = D // P            # 8 d-tiles (contraction for projections)
    ET = D // P            # 8 e-tiles
    JT = S // P            # 16 j-tiles
    JC = S // 512          # 4 j-chunks
    IG = NQ // 512         # 2 i-groups
    EC = D // 512          # 2 e-chunks

    with tile.TileContext(nc) as tc, ExitStack() as ctx:
        consts = ctx.enter_context(tc.tile_pool(name="consts", bufs=1))
        ps512 = ctx.enter_context(tc.tile_pool(name="ps512", bufs=2, space="PSUM"))
        outps = ctx.enter_context(tc.tile_pool(name="outps", bufs=2, space="PSUM"))
        qpool = ctx.enter_context(tc.tile_pool(name="qpool", bufs=1))
        kpool = ctx.enter_context(tc.tile_pool(name="kpool", bufs=1))
        vpool = ctx.enter_context(tc.tile_pool(name="vpool", bufs=1))

        _eng = [nc.sync, nc.gpsimd]
        _dmac = [0]
        def dma(out_ap, in_ap):
            e = _eng[_dmac[0] % len(_eng)]
            _dmac[0] += 1
            e.dma_start(out_ap, in_ap)

        # ---- constants (DMAs on the scalar queue so they don't queue
        # behind the bulk weight/x loads) ----
        ones_f = consts.tile([P, 1], F32)
        nc.gpsimd.memset(ones_f[:], 1.0)
        ones_col = consts.tile([P, 1], BF)
        nc.vector.tensor_copy(ones_col[:], ones_f[:])

        nbias = consts.tile([P, 1], F32)
        nc.gpsimd.memset(nbias[:], -2.5)

        # PE warm-up: dense dummy matmuls (no DMA deps) during the initial
        # input-load window so the HAM clock gate reaches 2.4 GHz before the
        # real matmuls start.
        warm_f = consts.tile([P, 512], F32)
        nc.gpsimd.memset(warm_f[:], 0.0)
        warm_bf = consts.tile([P, 512], BF)
        nc.vector.tensor_copy(warm_bf[:], warm_f[:])
        warm_sink = consts.tile([1, 1], F32)
        for w in range(36):
            wps = ps512.tile([P, 512], F32, tag="ps512")
            nc.tensor.matmul(wps[:], warm_bf[:, 0:P], warm_bf[:],
                             start=True, stop=True)
            if w == 35:
                nc.any.tensor_copy(warm_sink[:], wps[0:1, 0:1])

        ones_row_f = consts.tile([1, P], F32)
        nc.gpsimd.memset(ones_row_f[:], 1.0)
        ones_row = consts.tile([1, P], FR)
        nc.gpsimd.dma_start(ones_row[:], ones_row_f[:])
        bv_sb = consts.tile([1, D], FR)
        nc.gpsimd.dma_start(bv_sb[:], bv[:].rearrange("(one d) -> one d", one=1))
        bq_sb = consts.tile([P, ET], F32)
        nc.scalar.dma_start(bq_sb[:], bq[:].rearrange("(t p) -> p t", p=P))
        bk_sb = consts.tile([P, ET], F32)
        nc.scalar.dma_start(bk_sb[:], bk[:].rearrange("(t p) -> p t", p=P))

        # bv broadcast to [P, D] via ones.T @ bv (K=1 matmul)
        bv_bcast = consts.tile([P, D], F32)
        for ec in range(EC):
            pstmp = ps512.tile([P, 512], F32, tag="ps512")
            nc.tensor.matmul(pstmp[:], ones_row[:], bv_sb[:, ec * 512:(ec + 1) * 512],
                             start=True, stop=True)
            nc.any.tensor_copy(bv_bcast[:, ec * 512:(ec + 1) * 512], pstmp[:])

        # fp8 with paired contraction layout for DoubleRow matmuls:
        # global index = partition + 128*pair + 256*tile
        qT_sb = qpool.tile([P, ET // 2, 2, NQ], F8)   # [e-part, e-tile2, e-pair, i]
        kT_sb = kpool.tile([P, ET // 2, 2, S], F8)    # [e-part, e-tile2, e-pair, j]
        vN_sb = vpool.tile([P, JT, D], BF)            # [j-part, j-tile, e]

        # ---- projections ----
        with tc.tile_pool(name="wq", bufs=1) as wqp, \
             tc.tile_pool(name="wk", bufs=1) as wkp, \
             tc.tile_pool(name="wv", bufs=1) as wvp, \
             tc.tile_pool(name="xp", bufs=1) as xp:
            wq_sb = wqp.tile([P, DT, D], BF)
            wk_sb = wkp.tile([P, DT, D], BF)
            wv_sb = wvp.tile([P, DT, D], BF)
            xT_sb = xp.tile([P, DT, S], BF)
            # DMA issue order: Q-path first so the PE can start ASAP.
            for dt in range(DT):
                dma(wq_sb[:, dt, :], wqT[dt * P:(dt + 1) * P, :])
                dma(xT_sb[:, dt, 0:NQ], xT[dt * P:(dt + 1) * P, 0:NQ])
            for dt in range(DT):
                dma(wk_sb[:, dt, :], wkT[dt * P:(dt + 1) * P, :])
                dma(xT_sb[:, dt, NQ:S], xT[dt * P:(dt + 1) * P, NQ:S])
            for dt in range(DT):
                dma(wv_sb[:, dt, :], wvT[dt * P:(dt + 1) * P, :])

            # Q projection: qT[e, i]
            for g in range(IG):
                for et in range(ET):
                    ps = ps512.tile([P, 512], F32, tag="ps512")
                    for dt in range(DT):
                        nc.tensor.matmul(ps[:], wq_sb[:, dt, et * P:(et + 1) * P],
                                         xT_sb[:, dt, g * 512:(g + 1) * 512],
                                         start=(dt == 0), stop=(dt == DT - 1))
                    nc.vector.tensor_scalar_add(
                        qT_sb[:, et // 2, et % 2, g * 512:(g + 1) * 512],
                        ps[:], bq_sb[:, et:et + 1])
            # K projection: kT[e, j]
            for jc in range(JC):
                for et in range(ET):
                    ps = ps512.tile([P, 512], F32, tag="ps512")
                    for dt in range(DT):
                        nc.tensor.matmul(ps[:], wk_sb[:, dt, et * P:(et + 1) * P],
                                         xT_sb[:, dt, jc * 512:(jc + 1) * 512],
                                         start=(dt == 0), stop=(dt == DT - 1))
                    nc.vector.tensor_scalar_add(
                        kT_sb[:, et // 2, et % 2, jc * 512:(jc + 1) * 512],
                        ps[:], bk_sb[:, et:et + 1])
            # V projection: v[j, e] with bv added during copyback (softmax
            # weights sum to 1, so out = attn@v + bv == attn@(v + bv))
            for jt in range(JT):
                for ec in range(EC):
                    ps = ps512.tile([P, 512], F32, tag="ps512")
                    for dt in range(DT):
                        nc.tensor.matmul(ps[:], xT_sb[:, dt, jt * P:(jt + 1) * P],
                                         wv_sb[:, dt, ec * 512:(ec + 1) * 512],
                                         start=(dt == 0), stop=(dt == DT - 1))
                    nc.vector.tensor_add(vN_sb[:, jt, ec * 512:(ec + 1) * 512],
                                         ps[:], bv_bcast[:, ec * 512:(ec + 1) * 512])

        # ---- attention ----
        with tc.tile_pool(name="attn", bufs=2) as attnp, \
             tc.tile_pool(name="epi", bufs=2) as epip:
            for g in range(IG):
                aT = attnp.tile([P, JT, 512], BF, tag="aT")   # [j-part, j-tile, i]
                for jt in range(JT):
                    ps = ps512.tile([P, 512], F32, tag="ps512")
                    for et2 in range(ET // 2):
                        nc.tensor.matmul(ps[:], kT_sb[:, et2, :, jt * P:(jt + 1) * P],
                                         qT_sb[:, et2, :, g * 512:(g + 1) * 512],
                                         start=(et2 == 0), stop=(et2 == ET // 2 - 1),
                                         perf_mode=DR)
                    # exp(x/32 - 2.5): constant shift is softmax-invariant
                    nc.scalar.activation(aT[:, jt, :], ps[:], AF.Exp,
                                         scale=INV_SQRT_D, bias=nbias[:])
                for ib in range(4):
                    o0 = outps.tile([P, 512], F32, tag="o0")
                    o1 = outps.tile([P, 512], F32, tag="o1")
                    rs = outps.tile([P, 1], F32, tag="rs")
                    for jt in range(JT):
                        lhsT = aT[:, jt, ib * P:(ib + 1) * P]
                        st, sp = (jt == 0), (jt == JT - 1)
                        nc.tensor.matmul(o0[:], lhsT, vN_sb[:, jt, 0:512],
                                         start=st, stop=sp)
                        nc.tensor.matmul(o1[:], lhsT, vN_sb[:, jt, 512:1024],
                                         start=st, stop=sp)
                        nc.tensor.matmul(rs[:], lhsT, ones_col[:],
                                         start=st, stop=sp)
                    inv = epip.tile([P, 1], F32, tag="inv")
                    nc.vector.reciprocal(inv[:], rs[:])
                    osb = epip.tile([P, D], F32, tag="osb")
                    r0 = g * 512 + ib * P
                    nc.vector.tensor_scalar_mul(osb[:, 0:512], o0[:], inv[:])
                    dma(out[r0:r0 + P, 0:512], osb[:, 0:512])
                    nc.vector.tensor_scalar_mul(osb[:, 512:1024], o1[:], inv[:])
                    dma(out[r0:r0 + P, 512:1024], osb[:, 512:1024])

    nc.compile()
    return nc


def make_in_maps(x, Wq, bq, Wk, bk, Wv, bv):
    x = np.asarray(x, np.float32)
    wqT = np.asarray(Wq, np.float32).T.astype(BF16)
    wkT = np.asarray(Wk, np.float32).T.astype(BF16)
    wvT = np.asarray(Wv, np.float32).T.astype(BF16)
    bq = np.ascontiguousarray(np.asarray(bq, np.float32))
    bk = np.ascontiguousarray(np.asarray(bk, np.float32))
    bv = np.ascontiguousarray(np.asarray(bv, np.float32))
    in_maps = []
    for c in range(NCORES):
        b, h = c // 2, c % 2
        xb = x[b]
        # rotate so this core's query half is columns 0:NQ of xT
        xrot = np.concatenate([xb[h * NQ:(h + 1) * NQ], xb[(1 - h) * NQ:(2 - h) * NQ]], axis=0)
        in_maps.append({
            "xT": xrot.T.astype(BF16),
            "wqT": wqT, "wkT": wkT, "wvT": wvT,
            "bq": bq, "bk": bk, "bv": bv,
        })
    return in_maps


def get_nc():
    if "nc" not in _CACHE:
        _CACHE["nc"] = build_nc()
    return _CACHE["nc"]


def kernel(x, Wq, bq, Wk, bk, Wv, bv):
    from concourse.bass_utils import run_bass_kernel_spmd
    nc = get_nc()
    in_maps = make_in_maps(x, Wq, bq, Wk, bk, Wv, bv)
    res = run_bass_kernel_spmd(nc, in_maps, core_ids=list(range(NCORES)))
    out = np.empty((B, S, D), np.float32)
    for c in range(NCORES):
        b, h = c // 2, c % 2
        out[b, h * NQ:(h + 1) * NQ] = res.results[c]["out"]
    return out


# revision 26
# speedup vs baseline: 1.5740x; 1.0439x over previous
"""Single-head attention (InterModalAttention) Bass kernel for 8 TRN2 cores.

Sharding: batch (4) x query-half (2) -> 8 cores. Each core computes K/V for
its batch element (full 2048-seq) and attention for its 1024 queries.

v2 layout strategy (all bf16 matmul operands, f32 PSUM accumulation):
  - Host pre-transposes x and weights and rotates x columns per core so the
    core's query half occupies columns 0:1024 of xT -- no separate xq input.
  - x, weights, qT, kT, vN all SBUF-resident; no DRAM roundtrips.
  - qT/kT computed as [e, s] tiles (lhsT=WT tile, rhs=xT tile); bias added
    per-partition during PSUM->SBUF copyback (f32 -> bf16).
  - v computed natural [s, e]; bias bv folded into the final epilogue.
  - scoresT [j, i] computed directly (lhsT=kT tile, rhs=qT): the exp output
    written by ACT to SBUF is already the lhsT for out = attnT.T @ v, so no
    PE transposes and no PSUM->SBUF attn copies are needed.
  - softmax row sums come from an extra N=1 matmul against a ones column,
    reusing the stationary attnT tile already loaded for the AV matmuls.
  - epilogue: out = psum * (1/rowsum) + bv (DVE, f32).
"""
import sys
import numpy as np
import ml_dtypes

for p in ("/opt/trn_rl_repo",):
    if p not in sys.path:
        sys.path.insert(0, p)

B, S, D = 4, 2048, 1024
NQ = 1024          # queries per core
NCORES = 8
P = 128
INV_SQRT_D = 1.0 / 32.0
BF16 = ml_dtypes.bfloat16

_CACHE = {}


def build_nc():
    from contextlib import ExitStack
    import concourse.mybir as mybir
    import concourse.tile as tile
    from concourse import bacc

    F32 = mybir.dt.float32
    FR = mybir.dt.float32r
    BF = mybir.dt.bfloat16
    F8 = mybir.dt.float8e4
    DR = mybir.MatmulPerfMode.DoubleRow
    AF = mybir.ActivationFunctionType

    nc = bacc.Bacc("TRN2", debug=False)

    xT = nc.dram_tensor("xT", (D, S), BF, kind="ExternalInput")
    xN = nc.dram_tensor("xN", (S, D), BF, kind="ExternalInput")
    wqT = nc.dram_tensor("wqT", (D, D), BF, kind="ExternalInput")
    wkT = nc.dram_tensor("wkT", (D, D), BF, kind="ExternalInput")
    wvT = nc.dram_tensor("wvT", (D, D), BF, kind="ExternalInput")
    bq = nc.dram_tensor("bq", (D,), F32, kind="ExternalInput")
    bk = nc.dram_tensor("bk", (D,), F32, kind="ExternalInput")
    bv = nc.dram_tensor("bv", (D,), F32, kind="ExternalInput")
    out = nc.dram_tensor("out", (NQ, D), F32, kind="ExternalOutput")

    DT = D // P            # 8 d-tiles (contraction for projections)
    ET = D // P            # 8 e-tiles
    JT = S // P            # 16 j-tiles
    JC = S // 512          # 4 j-chunks
    IG = NQ // 512         # 2 i-groups
    EC = D // 512          # 2 e-chunks

    with tile.TileContext(nc) as tc, ExitStack() as ctx:
        consts = ctx.enter_context(tc.tile_pool(name="consts", bufs=1))
        ps512 = ctx.enter_context(tc.tile_pool(name="ps512", bufs=2, space="PSUM"))
        outps = ctx.enter_context(tc.tile_pool(name="outps", bufs=2, space="PSUM"))
        rsps = ctx.enter_context(tc.tile_pool(name="rsps", bufs=1, space="PSUM"))
        qpool = ctx.enter_context(tc.tile_pool(name="qpool", bufs=1))
        kpool = ctx.enter_context(tc.tile_pool(name="kpool", bufs=1))
        xnpool = ctx.enter_context(tc.tile_pool(name="xnpool", bufs=1))
        wvpool = ctx.enter_context(tc.tile_pool(name="wvpool", bufs=1))

        _eng = [nc.sync, nc.gpsimd]
        _dmac = [0]
        def dma(out_ap, in_ap):
            e = _eng[_dmac[0] % len(_eng)]
            _dmac[0] += 1
            e.dma_start(out_ap, in_ap)

        # ---- constants (DMAs on the scalar queue so they don't queue
        # behind the bulk weight/x loads) ----
        ones_f = consts.tile([P, 1], F32)
        nc.gpsimd.memset(ones_f[:], 1.0)
        ones_col = consts.tile([P, 1], BF)
        nc.vector.tensor_copy(ones_col[:], ones_f[:])

        nbias = consts.tile([P, 1], F32)
        nc.gpsimd.memset(nbias[:], -2.5)

        # PE warm-up: dense dummy matmuls (no DMA deps) during the initial
        # input-load window so the HAM clock gate reaches 2.4 GHz before the
        # real matmuls start.
        warm_f = consts.tile([P, 512], F32)
        nc.gpsimd.memset(warm_f[:], 0.0)
        warm_bf = consts.tile([P, 512], BF)
        nc.vector.tensor_copy(warm_bf[:], warm_f[:])
        warm_sink = consts.tile([1, 1], F32)
        for w in range(36):
            wps = ps512.tile([P, 512], F32, tag="ps512")
            nc.tensor.matmul(wps[:], warm_bf[:, 0:P], warm_bf[:],
                             start=True, stop=True)
            if w == 35:
                nc.any.tensor_copy(warm_sink[:], wps[0:1, 0:1])

        ones_row_f = consts.tile([1, P], F32)
        nc.gpsimd.memset(ones_row_f[:], 1.0)
        ones_row = consts.tile([1, P], FR)
        nc.gpsimd.dma_start(ones_row[:], ones_row_f[:])
        bv_sb = consts.tile([1, D], FR)
        nc.gpsimd.dma_start(bv_sb[:], bv[:].rearrange("(one d) -> one d", one=1))
        bq_sb = consts.tile([P, ET], F32)
        nc.scalar.dma_start(bq_sb[:], bq[:].rearrange("(t p) -> p t", p=P))
        bk_sb = consts.tile([P, ET], F32)
        nc.scalar.dma_start(bk_sb[:], bk[:].rearrange("(t p) -> p t", p=P))

        # bv broadcast to [P, D] via ones.T @ bv (K=1 matmul)
        bv_bcast = consts.tile([P, D], F32)
        for ec in range(EC):
            pstmp = ps512.tile([P, 512], F32, tag="ps512")
            nc.tensor.matmul(pstmp[:], ones_row[:], bv_sb[:, ec * 512:(ec + 1) * 512],
                             start=True, stop=True)
            nc.any.tensor_copy(bv_bcast[:, ec * 512:(ec + 1) * 512], pstmp[:])

        # fp8 with paired contraction layout for DoubleRow matmuls:
        # global index = partition + 128*pair + 256*tile
        qT_sb = qpool.tile([P, ET // 2, 2, NQ], F8)   # [e-part, e-tile2, e-pair, i]
        kT_sb = kpool.tile([P, ET // 2, 2, S], F8)    # [e-part, e-tile2, e-pair, j]
        xn_sb = xnpool.tile([P, JT, D], BF)           # x natural [j-part, j-tile, d]
        wv_sb = wvpool.tile([P, DT, D], BF)           # stays resident for the y-step

        # ---- projections ----
        with tc.tile_pool(name="wq", bufs=1) as wqp, \
             tc.tile_pool(name="wk", bufs=1) as wkp, \
             tc.tile_pool(name="xp", bufs=1) as xp:
            wq_sb = wqp.tile([P, DT, D], BF)
            wk_sb = wkp.tile([P, DT, D], BF)
            xT_sb = xp.tile([P, DT, S], BF)
            # DMA issue order: Q-path first so the PE can start ASAP.
            for dt in range(DT):
                dma(wq_sb[:, dt, :], wqT[dt * P:(dt + 1) * P, :])
                dma(xT_sb[:, dt, 0:NQ], xT[dt * P:(dt + 1) * P, 0:NQ])
            for dt in range(DT):
                dma(wk_sb[:, dt, :], wkT[dt * P:(dt + 1) * P, :])
                dma(xT_sb[:, dt, NQ:S], xT[dt * P:(dt + 1) * P, NQ:S])
            for dt in range(DT):
                dma(wv_sb[:, dt, :], wvT[dt * P:(dt + 1) * P, :])
            for jt in range(JT):
                dma(xn_sb[:, jt, :], xN[jt * P:(jt + 1) * P, :])

            # Q projection: qT[e, i]
            for g in range(IG):
                for et in range(ET):
                    ps = ps512.tile([P, 512], F32, tag="ps512")
                    for dt in range(DT):
                        nc.tensor.matmul(ps[:], wq_sb[:, dt, et * P:(et + 1) * P],
                                         xT_sb[:, dt, g * 512:(g + 1) * 512],
                                         start=(dt == 0), stop=(dt == DT - 1))
                    nc.vector.tensor_scalar_add(
                        qT_sb[:, et // 2, et % 2, g * 512:(g + 1) * 512],
                        ps[:], bq_sb[:, et:et + 1])
            # K projection: kT[e, j]
            for jc in range(JC):
                for et in range(ET):
                    ps = ps512.tile([P, 512], F32, tag="ps512")
                    for dt in range(DT):
                        nc.tensor.matmul(ps[:], wk_sb[:, dt, et * P:(et + 1) * P],
                                         xT_sb[:, dt, jc * 512:(jc + 1) * 512],
                                         start=(dt == 0), stop=(dt == DT - 1))
                    nc.vector.tensor_scalar_add(
                        kT_sb[:, et // 2, et % 2, jc * 512:(jc + 1) * 512],
                        ps[:], bk_sb[:, et:et + 1])
        # ---- attention (V projection is folded behind the attention matmul:
        # out = (attn @ x) @ WvT + bv, so only the 1024-row z needs projecting
        # instead of the 2048-row x) ----
        with tc.tile_pool(name="attn", bufs=2) as attnp, \
             tc.tile_pool(name="zt", bufs=2) as ztp, \
             tc.tile_pool(name="epi", bufs=2) as epip:
            for g in range(IG):
                aT = attnp.tile([P, JT, 512], BF, tag="aT")   # [j-part, j-tile, i]
                for jt in range(JT):
                    ps = ps512.tile([P, 512], F32, tag="ps512")
                    for et2 in range(ET // 2):
                        nc.tensor.matmul(ps[:], kT_sb[:, et2, :, jt * P:(jt + 1) * P],
                                         qT_sb[:, et2, :, g * 512:(g + 1) * 512],
                                         start=(et2 == 0), stop=(et2 == ET // 2 - 1),
                                         perf_mode=DR)
                    # exp(x/32 - 2.5): constant shift is softmax-invariant
                    nc.scalar.activation(aT[:, jt, :], ps[:], AF.Exp,
                                         scale=INV_SQRT_D, bias=nbias[:])
                # row sums of attn as a row vector: rs[i] = sum_j aT[j,i]
                rsr = rsps.tile([1, 512], F32, tag="rsrow")
                for jt in range(JT):
                    nc.tensor.matmul(rsr[:], ones_col[:], aT[:, jt, :],
                                     start=(jt == 0), stop=(jt == JT - 1))
                rsr_sb = epip.tile([1, 512], F32, tag="rsr_sb")
                nc.vector.tensor_copy(rsr_sb[:], rsr[:])
                # broadcast across partitions via K=1 matmul, then reciprocal
                rsb = rsps.tile([P, 512], F32, tag="rsb")
                nc.tensor.matmul(rsb[:], ones_row_f[:], rsr_sb[:],
                                 start=True, stop=True)
                invb = epip.tile([P, 512], F32, tag="invb")
                nc.vector.reciprocal(invb[:], rsb[:])
                # z-step: zT[d, i] = sum_j x[j, d] * aT[j, i], normalized by
                # 1/rowsum during PSUM->SBUF copyback
                zT = ztp.tile([P, DT, 512], BF, tag="zT")    # [d-part, d-tile, i]
                for dt in range(DT):
                    zps = ps512.tile([P, 512], F32, tag="ps512")
                    for jt in range(JT):
                        nc.tensor.matmul(zps[:], xn_sb[:, jt, dt * P:(dt + 1) * P],
                                         aT[:, jt, :],
                                         start=(jt == 0), stop=(jt == JT - 1))
                    nc.vector.tensor_mul(zT[:, dt, :], zps[:], invb[:])
                # y-step: out[i, e] = sum_d zT[d, i] * wvT[d, e] + bv
                for ib in range(4):
                    r0 = g * 512 + ib * P
                    for ec in range(EC):
                        yps = outps.tile([P, 512], F32, tag=f"o{ec}")
                        for dt in range(DT):
                            nc.tensor.matmul(yps[:], zT[:, dt, ib * P:(ib + 1) * P],
                                             wv_sb[:, dt, ec * 512:(ec + 1) * 512],
                                             start=(dt == 0), stop=(dt == DT - 1))
                        osb = epip.tile([P, 512], F32, tag=f"osb{ec}")
                        nc.vector.tensor_add(osb[:], yps[:],
                                             bv_bcast[:, ec * 512:(ec + 1) * 512])
                        dma(out[r0:r0 + P, ec * 512:(ec + 1) * 512], osb[:])

    nc.compile()
    return nc


def make_in_maps(x, Wq, bq, Wk, bk, Wv, bv):
    x = np.asarray(x, np.float32)
    wqT = np.asarray(Wq, np.float32).T.astype(BF16)
    wkT = np.asarray(Wk, np.float32).T.astype(BF16)
    wvT = np.asarray(Wv, np.float32).T.astype(BF16)
    bq = np.ascontiguousarray(np.asarray(bq, np.float32))
    bk = np.ascontiguousarray(np.asarray(bk, np.float32))
    bv = np.ascontiguousarray(np.asarray(bv, np.float32))
    in_maps = []
    for c in range(NCORES):
        b, h = c // 2, c % 2
        xb = x[b]
        # rotate so this core's query half is columns 0:NQ of xT
        xrot = np.concatenate([xb[h * NQ:(h + 1) * NQ], xb[(1 - h) * NQ:(2 - h) * NQ]], axis=0)
        in_maps.append({
            "xT": xrot.T.astype(BF16),
            "xN": xrot.astype(BF16),
            "wqT": wqT, "wkT": wkT, "wvT": wvT,
            "bq": bq, "bk": bk, "bv": bv,
        })
    return in_maps


def get_nc():
    if "nc" not in _CACHE:
        _CACHE["nc"] = build_nc()
    return _CACHE["nc"]


def kernel(x, Wq, bq, Wk, bk, Wv, bv):
    from concourse.bass_utils import run_bass_kernel_spmd
    nc = get_nc()
    in_maps = make_in_maps(x, Wq, bq, Wk, bk, Wv, bv)
    res = run_bass_kernel_spmd(nc, in_maps, core_ids=list(range(NCORES)))
    out = np.empty((B, S, D), np.float32)
    for c in range(NCORES):
        b, h = c // 2, c % 2
        out[b, h * NQ:(h + 1) * NQ] = res.results[c]["out"]
    return out
